# revision 7
# baseline (speedup 1.0000x reference)
"""Hadamard transform kernel for Trainium2 (8 NeuronCores, SPMD).

Problem: x (8192, 4096) fp32; apply a 128-point Hadamard transform to each
contiguous 128-element group of every row.  Equivalent to
    out = (x.reshape(-1, 128) @ M).reshape(8192, 4096)
where M is the 128x128 butterfly matrix (symmetric, entries +/- 2^-3.5).

bf16 end-to-end (tolerance is 2e-2; bf16 error is ~3e-3):
  - Host pre-scales x by sqrt(2) and casts to bf16; the device matrix is the
    raw +/-1 Hadamard scaled by 2^-4 (exact in bf16), so combined scaling is
    exactly H/sqrt(128).  Halves HBM traffic (DMA floor ~47us/core) and
    quadruples PE throughput vs fp32.
  - Host upcasts the bf16 result back to fp32.

Data flow per core (1024 rows):
  - Input is viewed flat as [32768, 128]: row k is the k-th 128-element
    group (contiguous 512B->256B in bf16).  Each tile of 4096 consecutive
    groups (= 128 rows x 32 groups, 1 MiB) is loaded with the hardware
    DMA transpose (xbar) straight into SBUF as [128, 4096] with the
    within-group element index on partitions -- the fully-contiguous
    source is the fast xbar case, and it removes the PE-transpose pass
    and its PSUM->SBUF copy entirely.
  - For group g of the tile, lhsT = xT[:, g::32] (a [128,128] strided
    slice whose free axis enumerates the tile's 128 rows), so
    matmul(lhsT, rhs=M) -> PSUM gives the transformed block already in
    natural [row, coeff] orientation.
  - PSUM fp32 -> SBUF bf16 evacuation alternates scalar/vector engines;
    output tiles [128, 4096] stream back on the scalar HWDGE ring.
"""

import math

import numpy as np
import ml_dtypes

import concourse.bass as bass
import concourse.tile as tile
from concourse import bacc, mybir
from concourse.bass import ts
from concourse.bass_utils import run_bass_kernel_spmd

N_CORES = 8
ROWS, COLS = 8192, 4096
R_CORE = ROWS // N_CORES  # 1024 rows per core
G = 128                   # hadamard group size
NG = COLS // G            # 32 groups per row
NGC = R_CORE * NG         # 32768 groups per core
TG = 128 * NG             # 4096 groups per tile (= 128 rows)
NT = NGC // TG            # 8 tiles per core

BF16 = ml_dtypes.bfloat16


def _hadamard_raw() -> np.ndarray:
    """Raw +/-1 Sylvester Hadamard matrix of order 128 (symmetric)."""
    h = np.array([[1.0]], dtype=np.float64)
    for _ in range(int(math.log2(G))):
        h = np.block([[h, h], [h, -h]])
    return h


def _build_module():
    nc = bacc.Bacc("TRN2", target_bir_lowering=False, debug=False)
    bf16 = mybir.dt.bfloat16
    f32 = mybir.dt.float32
    x_d = nc.dram_tensor("x", [NGC, G], bf16, kind="ExternalInput")
    h_d = nc.dram_tensor("hmat", [G, G], bf16, kind="ExternalInput")
    o_d = nc.dram_tensor("out", [R_CORE, COLS], bf16, kind="ExternalOutput")

    with tile.TileContext(nc) as tc:
        with (
            tc.tile_pool(name="const", bufs=1) as cpool,
            tc.tile_pool(name="xin", bufs=3) as xpool,
            tc.tile_pool(name="outb", bufs=3) as opool,
            tc.tile_pool(name="pst", bufs=1, space=bass.MemorySpace.PSUM) as pst,
            tc.tile_pool(name="psm", bufs=6, space=bass.MemorySpace.PSUM) as psm,
        ):
            # PE warmup: dummy transposes with no data deps so the PE's
            # HAM clock-gate opens during the initial DMA wait.  (Padded
            # to a full 2 KiB PSUM bank so no other buf shares the bank.)
            wsb = cpool.tile([G, G], bf16)
            nc.gpsimd.memset(wsb[:], 1.0)
            wp = pst.tile([G, G], bf16, tag="pt", padded_shape=[128, 1024])
            for _ in range(26):
                nc.tensor.transpose(wp[:, :G], wsb[:], wsb[:])

            hm = cpool.tile([G, G], bf16)
            nc.sync.dma_start(hm[:], h_d[:])

            for t in range(NT):
                xt = xpool.tile([128, TG], bf16, tag="xt")
                nc.sync.dma_start(
                    xt[:], x_d[t * TG:(t + 1) * TG, :], transpose=True
                )
                ot = opool.tile([128, COLS], bf16, tag="ot")
                for q in range(NG // 4):
                    pm = psm.tile([128, 512], f32)
                    for j in range(4):
                        g = q * 4 + j
                        nc.tensor.matmul(
                            pm[:, ts(j, G)], xt[:, g::NG], hm[:]
                        )
                    # whole-tile evacuation (reads depend on all four
                    # matmuls -> no same-bank read-while-PE-writes),
                    # alternating engines to balance load.
                    if q % 2 == 0:
                        nc.scalar.copy(ot[:, ts(q, 512)], pm[:])
                    else:
                        nc.vector.tensor_copy(ot[:, ts(q, 512)], pm[:])
                nc.scalar.dma_start(
                    o_d[t * 128:(t + 1) * 128, :], ot[:]
                )

    nc.compile()
    return nc


_NC_CACHE = None


def _get_nc():
    global _NC_CACHE
    if _NC_CACHE is None:
        _NC_CACHE = _build_module()
    return _NC_CACHE


def _in_maps(x: np.ndarray) -> list:
    """Shard + bf16-encode the full fp32 input for the 8 cores."""
    xs = np.asarray(x, dtype=np.float32) * np.float32(math.sqrt(2.0))
    xb = xs.astype(BF16)
    hmat = (_hadamard_raw() * 0.0625).astype(BF16)
    return [
        {
            "x": np.ascontiguousarray(
                xb[c * R_CORE:(c + 1) * R_CORE].reshape(NGC, G)
            ),
            "hmat": hmat,
        }
        for c in range(N_CORES)
    ]


def kernel(x) -> np.ndarray:
    assert x.shape == (ROWS, COLS)
    nc = _get_nc()
    res = run_bass_kernel_spmd(nc, _in_maps(x), core_ids=list(range(N_CORES)))
    out = np.concatenate([r["out"] for r in res.results], axis=0)
    return out.astype(np.float32)


# revision 8
# speedup vs baseline: 1.5828x; 1.5828x over previous
"""Hadamard transform kernel for Trainium2 (8 NeuronCores, SPMD).

Problem: x (8192, 4096) fp32; apply a 128-point Hadamard transform to each
contiguous 128-element group of every row.  Equivalent to
    out = (x.reshape(-1, 128) @ M).reshape(8192, 4096)
where M is the 128x128 butterfly matrix (symmetric, entries +/- 2^-3.5).

bf16 end-to-end (tolerance is 2e-2; bf16 error is ~3e-3):
  - Host pre-scales x by sqrt(2) and casts to bf16; the device matrix is the
    raw +/-1 Hadamard scaled by 2^-4 (exact in bf16), so combined scaling is
    exactly H/sqrt(128).  Halves HBM traffic and quadruples PE throughput
    vs fp32.  Host upcasts the bf16 result back to fp32.

Layout (the host owns both en/decode, so the device sees transposed blocks):
  - Host sends x_dev[c, (t, g, r)] = x[t*128 + r, g*128 + c] per core: the
    within-group element index c on partitions, groups g major in the free
    dim.  Per 512-wide quad (4 groups x 128 rows) ONE matmul with the
    stationary Hadamard matrix computes M @ x^T = (x @ M)^T, i.e. 64
    matmuls of N=512 per core and zero on-chip transposes.
  - PSUM fp32 -> SBUF bf16 evacuation alternates scalar/vector engines;
    the output goes back in the same transposed layout and the host
    permutes it to natural orientation.
  - DMA chunks cover two 128-row tiles -> 16 KiB contiguous per-partition
    lines, 2 MiB per transfer (near peak DMA efficiency); first/last
    chunks are halved to shorten pipeline fill/drain.
"""

import math

import numpy as np
import ml_dtypes

import concourse.bass as bass
import concourse.tile as tile
from concourse import bacc, mybir
from concourse.bass import ts
from concourse.bass_utils import run_bass_kernel_spmd

N_CORES = 8
ROWS, COLS = 8192, 4096
R_CORE = ROWS // N_CORES  # 1024 rows per core
G = 128                   # hadamard group size
NG = COLS // G            # 32 groups per row
NGC = R_CORE * NG         # 32768 groups per core
NT = R_CORE // 128        # 8 row-tiles per core (4096 free elems each)

BF16 = ml_dtypes.bfloat16

# free-dim chunking (in elements of the [128, 32768] device view):
# 1 MiB head/tail chunks, 2 MiB (two-tile) middle chunks
CHUNKS = [4096, 8192, 8192, 8192, 4096]
assert sum(CHUNKS) == NGC


def _hadamard_raw() -> np.ndarray:
    """Raw +/-1 Sylvester Hadamard matrix of order 128 (symmetric)."""
    h = np.array([[1.0]], dtype=np.float64)
    for _ in range(int(math.log2(G))):
        h = np.block([[h, h], [h, -h]])
    return h


def _build_module():
    nc = bacc.Bacc("TRN2", target_bir_lowering=False, debug=False)
    bf16 = mybir.dt.bfloat16
    f32 = mybir.dt.float32
    x_d = nc.dram_tensor("x", [G, NGC], bf16, kind="ExternalInput")
    h_d = nc.dram_tensor("hmat", [G, G], bf16, kind="ExternalInput")
    o_d = nc.dram_tensor("out", [G, NGC], bf16, kind="ExternalOutput")

    with tile.TileContext(nc) as tc:
        with (
            tc.tile_pool(name="const", bufs=1) as cpool,
            tc.tile_pool(name="xin", bufs=3) as xpool,
            tc.tile_pool(name="outb", bufs=3) as opool,
            tc.tile_pool(name="pst", bufs=1, space=bass.MemorySpace.PSUM) as pst,
            tc.tile_pool(name="psm", bufs=6, space=bass.MemorySpace.PSUM) as psm,
        ):
            # PE warmup: dummy transposes with no data deps so the PE's
            # HAM clock-gate opens during the initial DMA wait.  (Padded
            # to a full 2 KiB PSUM bank so no other buf shares the bank.)
            wsb = cpool.tile([G, G], bf16)
            nc.gpsimd.memset(wsb[:], 1.0)
            wp = pst.tile([G, G], bf16, tag="pt", padded_shape=[128, 1024])
            for _ in range(26):
                nc.tensor.transpose(wp[:, :G], wsb[:], wsb[:])

            hm = cpool.tile([G, G], bf16)
            nc.sync.dma_start(hm[:], h_d[:])

            c0 = 0
            for cc in CHUNKS:
                xt = xpool.tile([128, cc], bf16, tag="xt")
                nc.sync.dma_start(xt[:], x_d[:, c0:c0 + cc])
                ot = opool.tile([128, cc], bf16, tag="ot")
                for q in range(cc // 512):
                    pm = psm.tile([128, 512], f32)
                    # one matmul per quad: stationary Hadamard, 512
                    # moving columns -> (x @ M)^T for 4 groups at once
                    nc.tensor.matmul(pm[:], hm[:], xt[:, ts(q, 512)])
                    if q % 2 == 0:
                        nc.scalar.copy(ot[:, ts(q, 512)], pm[:])
                    else:
                        nc.vector.tensor_copy(ot[:, ts(q, 512)], pm[:])
                nc.scalar.dma_start(o_d[:, c0:c0 + cc], ot[:])
                c0 += cc

    nc.compile()
    return nc


_NC_CACHE = None


def _get_nc():
    global _NC_CACHE
    if _NC_CACHE is None:
        _NC_CACHE = _build_module()
    return _NC_CACHE


def _in_maps(x: np.ndarray) -> list:
    """Shard, bf16-encode and block-transpose the input for the 8 cores."""
    xs = np.asarray(x, dtype=np.float32) * np.float32(math.sqrt(2.0))
    xb = xs.astype(BF16)
    hmat = (_hadamard_raw() * 0.0625).astype(BF16)
    maps = []
    for c in range(N_CORES):
        shard = xb[c * R_CORE:(c + 1) * R_CORE]          # [1024, 4096]
        dev = shard.reshape(NT, 128, NG, G)              # [t, r, g, c]
        dev = dev.transpose(3, 0, 2, 1).reshape(G, NGC)  # [c, (t, g, r)]
        maps.append({"x": np.ascontiguousarray(dev), "hmat": hmat})
    return maps


def _decode_out(o_dev: np.ndarray) -> np.ndarray:
    """Inverse of the block-transposed layout: [j, (t, g, r)] -> natural."""
    o = o_dev.reshape(G, NT, NG, 128)        # [j, t, g, r]
    return np.ascontiguousarray(
        o.transpose(1, 3, 2, 0).reshape(R_CORE, COLS)
    )


def kernel(x) -> np.ndarray:
    assert x.shape == (ROWS, COLS)
    nc = _get_nc()
    res = run_bass_kernel_spmd(nc, _in_maps(x), core_ids=list(range(N_CORES)))
    out = np.concatenate(
        [_decode_out(r["out"]) for r in res.results], axis=0
    )
    return out.astype(np.float32)
